# revision 2
# baseline (speedup 1.0000x reference)
"""BlackwellLinear Trainium2 kernel: 2:4 sparsity + int8 fake-quant + x @ w.T + bias.

Full inputs in, full output out. Data-parallel over tokens across 8 NeuronCores;
weight/bias replicated. All module math (sparsify, quantize, matmul, bias) runs
on device; the host only re-encodes layouts: x is transposed to fp16, and the
in_features axis of both x.T and w.T is permuted phase-major
(p <-> 4*(p%256) + p//256). The permutation makes each group-of-4 (the 2:4
sparsity unit) span four k-tiles at the SAME partition/column coordinates, so
the whole sparsify+quantize pipeline is contiguous full-width elementwise ops
and the quantized weight is produced directly in [in_f, out_f] (lhsT) layout --
no on-device transposes. A contraction-axis permutation applied to both
operands leaves the matmul result unchanged.

Numerics (target: harness gate rel_err < 2e-2; this kernel lands ~1e-3):
  s   = absmax * (1/qmax)                  (1 ulp from fl(absmax/qmax))
  inv ~= 1/s                               (HW reciprocal + 1 Newton step)
  k   = rne(w * inv)                       (magic-constant RNE round on ACT)
  q   = k * mask                           (2:4 mask, fp32 -> fp16 exact)
  y   = s * (x16 @ q.T) + bias             (scale folded into PSUM eviction)
x is sent as fp16 (rel err 2^-11); q <= 127 is fp16-exact; products are exact
in fp32 PSUM. One fp16 PE pass = 1 cycle/row -- the dense-matmul roofline.
The 2:4 threshold/mask math runs on fp32 weights so near-tie group selections
match the fp32 reference exactly.
"""

import numpy as np

N_CORES = 8
P = 128
IN_F = 1024
OUT_F = 1024
TOKENS = 32768
TOK_PER_CORE = TOKENS // N_CORES  # 4096
K_TILES = IN_F // P  # 8
M_TILES = OUT_F // P  # 8
TB_TOK = 1024  # token block per x strip
N_TB = TOK_PER_CORE // TB_TOK  # 4
MM_N = 512  # matmul moving free dim (one PSUM bank of fp32)
TJ = TB_TOK // MM_N  # matmuls per (mi, ki) stationary load

MAGIC = 12582912.0  # 1.5 * 2**23: (v + MAGIC) - MAGIC == RNE round for |v| <= 2**22

# k-tile processing order: range-0 tiles (phases of groups 0..127) first so the
# range-0 threshold -> masks -> quant chain completes with only half the weight
# DMA landed; PE consumes k-tiles in this same order (accumulation commutes).
KT_ORDER = (0, 2, 4, 6, 1, 3, 5, 7)

N_WARM = 0  # dummy matmuls to hold the PE HAM un-throttled before real work

# phase-major permutation of the in_features axis: position p holds original
# feature 4*(p%256) + p//256, so k-tile kt covers phase kt//2 of group range
# (kt%2)*128..+128 and the four phases of a group share partition/column coords
_PERM = (4 * (np.arange(IN_F) % 256) + np.arange(IN_F) // 256).astype(np.int64)

_CACHE = {}


def _build(qmax: float):
    from contextlib import ExitStack

    import concourse.tile as tile
    import concourse.mybir as mybir
    from concourse import bacc, bass_isa

    f32 = mybir.dt.float32
    f16 = mybir.dt.float16
    Alu = mybir.AluOpType
    Act = mybir.ActivationFunctionType

    inv_qmax = float(np.float32(1.0) / np.float32(qmax))

    nc = bacc.Bacc("TRN2", target_bir_lowering=False, debug=False)
    xth = nc.dram_tensor("xth", [IN_F, TOK_PER_CORE], f16, kind="ExternalInput").ap()
    # wp: w.T with permuted in_f rows = [in_f_perm, out_f], fp32
    wp = nc.dram_tensor("wp", [IN_F, OUT_F], f32, kind="ExternalInput").ap()
    bias = nc.dram_tensor("bias", [OUT_F], f32, kind="ExternalInput").ap()
    yt = nc.dram_tensor("yt", [OUT_F, TOK_PER_CORE], f16, kind="ExternalOutput").ap()

    with tile.TileContext(nc) as tc, ExitStack() as ctx:
        const = ctx.enter_context(tc.tile_pool(name="const", bufs=1))
        wnat_p = ctx.enter_context(tc.tile_pool(name="wnat", bufs=8))
        abs_p = ctx.enter_context(tc.tile_pool(name="absp", bufs=8))
        thr_p = ctx.enter_context(tc.tile_pool(name="thr", bufs=2))
        thrtmp_p = ctx.enter_context(tc.tile_pool(name="thrtmp", bufs=1))
        mask_p = ctx.enter_context(tc.tile_pool(name="mask", bufs=8))
        qtmp_p = ctx.enter_context(tc.tile_pool(name="qtmp", bufs=2))
        wqt_p = ctx.enter_context(tc.tile_pool(name="wqt", bufs=8))
        sc_p = ctx.enter_context(tc.tile_pool(name="sc", bufs=1))
        x_p = ctx.enter_context(tc.tile_pool(name="x", bufs=16))
        y_p = ctx.enter_context(tc.tile_pool(name="y", bufs=4))
        psum_mm = ctx.enter_context(tc.tile_pool(name="psmm", bufs=8, space="PSUM"))

        # ---- optional PE pre-warm: keep HAM at full clock until real MMs ----
        if N_WARM > 0:
            zwarm = const.tile([P, MM_N], f16, tag="zwarm")
            nc.gpsimd.memset(zwarm[:], 0.0)
            ps_w = psum_mm.tile([P, MM_N], f32, tag="ps", name="ps_warm")
            for _ in range(N_WARM):
                nc.tensor.matmul(
                    ps_w[:], zwarm[:, 0:P], zwarm[:], start=True, stop=True
                )

        # ---- weight load (split across both HWDGE queues for full BW) ----
        wk = [None] * K_TILES
        ak = [None] * K_TILES
        cm = sc_p.tile([P, 8], f32, tag="cm")
        for i, kt in enumerate(KT_ORDER):
            wt = wnat_p.tile([P, OUT_F], f32, tag="wnat", name=f"wnat{kt}")
            (nc.sync if i % 2 == 0 else nc.scalar).dma_start(
                wt[:], wp[kt * P : (kt + 1) * P, :]
            )
            wk[kt] = wt
            a = abs_p.tile([P, OUT_F], f32, tag="abs", name=f"abs{kt}")
            nc.scalar.activation(a[:], wt[:], Act.Abs)
            ak[kt] = a
            nc.vector.tensor_reduce(
                out=cm[:, i : i + 1],
                in_=a[:],
                axis=mybir.AxisListType.X,
                op=Alu.max,
            )

        # ---- bias slices (scalar queue; idle until y stores ~20us in) ----
        bias_t = []
        for mi in range(M_TILES):
            bt = const.tile([P, 1], f32, tag=f"bias{mi}")
            nc.scalar.dma_start(bt[:, 0:1], bias[mi * P : (mi + 1) * P].unsqueeze(1))
            bias_t.append(bt)

        # ---- global absmax broadcast to all partitions ----
        amc = sc_p.tile([P, 1], f32, tag="amc")
        nc.vector.reduce_max(amc[:], cm[:], axis=mybir.AxisListType.X)
        am = sc_p.tile([P, 1], f32, tag="am")
        nc.gpsimd.partition_all_reduce(
            am[:], amc[:], channels=P, reduce_op=bass_isa.ReduceOp.max
        )

        def vts(out, in0, s1, op0, s2=None, op1=None):
            kw = {"op1": op1} if op1 is not None else {}
            nc.vector.tensor_scalar(
                out=out, in0=in0, scalar1=s1, scalar2=s2, op0=op0, **kw
            )

        def vtt(out, in0, in1, op):
            nc.vector.tensor_tensor(out=out, in0=in0, in1=in1, op=op)

        # ---- s = absmax/qmax (1 ulp); inv = 1/s (reciprocal + 1 Newton) ----
        s_t = sc_p.tile([P, 1], f32, tag="s")
        vts(s_t[:], am[:], inv_qmax, Alu.mult)
        r0 = sc_p.tile([P, 1], f32, tag="r0")
        nc.vector.reciprocal(r0[:], s_t[:])
        p1 = sc_p.tile([P, 1], f32, tag="p1")
        vtt(p1[:], s_t[:], r0[:], Alu.mult)
        e1 = sc_p.tile([P, 1], f32, tag="e1")
        vts(e1[:], p1[:], 2.0, Alu.subtract, -1.0, Alu.mult)  # 2 - s*r0
        inv_t = sc_p.tile([P, 1], f32, tag="inv")
        vtt(inv_t[:], r0[:], e1[:], Alu.mult)

        magic_t = sc_p.tile([P, 1], f32, tag="magic")
        nc.gpsimd.memset(magic_t[:], MAGIC)

        # ---- 2:4 threshold per group-range (contiguous, phases = k-tiles) ----
        # thr_r = 2nd largest |w| of each group = max(min of pair maxes,
        # max of pair mins) over the 4 phase tiles of range r
        def build_thr(r):
            a0, a1, a2, a3 = (ak[2 * j + r] for j in range(4))
            tA = thrtmp_p.tile([P, OUT_F], f32, tag="tA", name=f"tA_{r}")
            tB = thrtmp_p.tile([P, OUT_F], f32, tag="tB", name=f"tB_{r}")
            tC = thrtmp_p.tile([P, OUT_F], f32, tag="tC", name=f"tC_{r}")
            tr = thr_p.tile([P, OUT_F], f32, tag="thr", name=f"thr_{r}")
            vtt(tA[:], a0[:], a1[:], Alu.max)
            vtt(tB[:], a2[:], a3[:], Alu.max)
            vtt(tA[:], tA[:], tB[:], Alu.min)  # t1 = min of pair maxes
            vtt(tB[:], a0[:], a1[:], Alu.min)
            vtt(tC[:], a2[:], a3[:], Alu.min)
            vtt(tB[:], tB[:], tC[:], Alu.max)  # t2 = max of pair mins
            vtt(tr[:], tA[:], tB[:], Alu.max)
            return tr

        # ---- per k-tile: k = rne(w*inv) on ACT (magic round), 2:4 mask on
        # gpsimd (parallel engine), combine+cast to fp16 lhsT tiles on DVE.
        # Masks don't depend on the global scale, so they run while the
        # absmax/scale chain resolves; quant follows as soon as inv lands.
        wqt_by_kt = {}
        thr_cache = {}
        for kt in KT_ORDER:
            r = kt % 2
            if r not in thr_cache:
                thr_cache[r] = build_thr(r)
            wt, a, tr = wk[kt], ak[kt], thr_cache[r]
            m = mask_p.tile([P, OUT_F], f32, tag="mask", name=f"m{kt}")
            nc.gpsimd.tensor_tensor(out=m[:], in0=a[:], in1=tr[:], op=Alu.is_ge)

            q0 = qtmp_p.tile([P, OUT_F], f32, tag="q0", name=f"q0_{kt}")
            # q0 = rne(w * inv) + MAGIC  (ACT: in*scale + bias)
            nc.scalar.activation(
                q0[:], wt[:], Act.Identity, bias=magic_t[:], scale=inv_t[:]
            )
            q16 = wqt_p.tile([P, OUT_F], f16, tag="q16", name=f"q16_{kt}")
            # q16 = (q0 - MAGIC) * mask, cast fp16 (exact: integer <= qmax)
            nc.vector.scalar_tensor_tensor(
                out=q16[:], in0=q0[:], scalar=-MAGIC, in1=m[:],
                op0=Alu.add, op1=Alu.mult,
            )
            wqt_by_kt[kt] = q16
        wqt = [wqt_by_kt[kt] for kt in range(K_TILES)]

        # ---- main matmul: yt[m, t] = sum_k wqt[k,m].T @ xh[k,t] ----
        # all x loads ride the sync queue (nothing else competes there);
        # y stores ride the scalar queue so evictions/stores pair up on ACT
        for tb in range(N_TB):
            xh = [None] * K_TILES
            for ki in KT_ORDER:
                sl_p = slice(ki * P, (ki + 1) * P)
                sl_t = slice(tb * TB_TOK, (tb + 1) * TB_TOK)
                xht = x_p.tile([P, TB_TOK], f16, tag="xh", name=f"xh{tb}_{ki}")
                nc.sync.dma_start(xht[:], xth[sl_p, sl_t])
                xh[ki] = xht

            def evict(mi, ps_tj):
                ysb = y_p.tile([P, TB_TOK], f16, tag="ysb", name=f"y{tb}_{mi}")
                for tj in range(TJ):
                    nc.scalar.activation(
                        ysb[:, tj * MM_N : (tj + 1) * MM_N],
                        ps_tj[tj][:],
                        Act.Identity,
                        bias=bias_t[mi][:],
                        scale=s_t[:],
                    )
                tcol = tb * TB_TOK
                nc.scalar.dma_start(
                    yt[mi * P : (mi + 1) * P, tcol : tcol + TB_TOK], ysb[:]
                )

            if tb == 0:
                # k-outer sweep: PE starts as soon as the first quantized
                # k-tile lands, consuming k-tiles at the prep pipeline's pace
                for mh in range(2):
                    ps = {
                        (ml, tj): psum_mm.tile(
                            [P, MM_N], f32, tag="ps", name=f"ps0_{mh}_{ml}_{tj}"
                        )
                        for ml in range(4)
                        for tj in range(TJ)
                    }
                    for kpos, ki in enumerate(KT_ORDER):
                        for ml in range(4):
                            mi = mh * 4 + ml
                            lhsT = wqt[ki][:, mi * P : (mi + 1) * P]
                            for tj in range(TJ):
                                nc.tensor.matmul(
                                    ps[ml, tj][:],
                                    lhsT,
                                    xh[ki][:, tj * MM_N : (tj + 1) * MM_N],
                                    start=(kpos == 0),
                                    stop=(kpos == K_TILES - 1),
                                )
                    for ml in range(4):
                        evict(mh * 4 + ml, [ps[ml, tj] for tj in range(TJ)])
            else:
                for mi in range(M_TILES):
                    ps = [
                        psum_mm.tile(
                            [P, MM_N], f32, tag="ps", name=f"ps{tb}_{mi}_{tj}"
                        )
                        for tj in range(TJ)
                    ]
                    for kpos, ki in enumerate(KT_ORDER):
                        lhsT = wqt[ki][:, mi * P : (mi + 1) * P]
                        for tj in range(TJ):
                            nc.tensor.matmul(
                                ps[tj][:],
                                lhsT,
                                xh[ki][:, tj * MM_N : (tj + 1) * MM_N],
                                start=(kpos == 0),
                                stop=(kpos == K_TILES - 1),
                            )
                    evict(mi, ps)

    nc.compile()
    return nc


def _get(qmax: float):
    key = qmax
    if key not in _CACHE:
        _CACHE[key] = _build(qmax)
    return _CACHE[key]


def host_prep(x, weight):
    """Host-side input re-encoding: transpose, phase-major permute the in_f
    axis, fp16-encode x. Pure layout/encoding; no module math."""
    xt = np.ascontiguousarray(x.T)[_PERM]  # [IN_F perm, TOKENS]
    xth = xt.astype(np.float16)
    wp = np.ascontiguousarray(weight.T[_PERM])  # [IN_F perm, OUT_F]
    return xth, wp


LAST_EXEC_NS = None


def kernel(x, weight, bias, precision, _trace_dir=None):
    global LAST_EXEC_NS
    from concourse.bass_utils import run_bass_kernel_spmd

    x = np.asarray(x, dtype=np.float32)
    weight = np.asarray(weight, dtype=np.float32)
    bias = np.asarray(bias, dtype=np.float32)
    prec = int(np.asarray(precision))
    qmax = float(2 ** (prec - 1) - 1)

    nc = _get(qmax)

    xth, wp = host_prep(x, weight)
    in_maps = [
        {
            "xth": np.ascontiguousarray(
                xth[:, c * TOK_PER_CORE : (c + 1) * TOK_PER_CORE]
            ),
            "wp": wp,
            "bias": bias,
        }
        for c in range(N_CORES)
    ]
    kw = {}
    if _trace_dir is not None:
        kw = {"trace": True, "tmpdir": _trace_dir}
    res = run_bass_kernel_spmd(nc, in_maps, list(range(N_CORES)), **kw)
    LAST_EXEC_NS = res.exec_time_ns
    yt = np.concatenate([res.results[c]["yt"] for c in range(N_CORES)], axis=1)
    return np.ascontiguousarray(yt.T).astype(np.float32)


# revision 4
# speedup vs baseline: 1.9940x; 1.9940x over previous
"""BlackwellLinear Trainium2 kernel: 2:4 sparsity + int8 fake-quant + x @ w.T + bias.

Full inputs in, full output out. Hybrid sharding across 8 NeuronCores:
4 token groups x 2 out_feature groups. Each core computes
y[tg-block, fg-block] = x[tg] @ w[fg].T * scale + bias[fg], and also runs the
module's weight prep (2:4 sparsify + int8 fake-quant) for its own out_feature
half -- halving the elementwise prep work per core vs pure data-parallel,
which is what gates the single-pass matmul pipeline start. No collectives:
the global absmax over the sparsified weight equals the absmax of |w| (the
global max always survives 2:4 top-2 selection), and each core computes it
from its own fp32 half plus a compact fp16 shadow of the other half (scale
perturbation ~2^-11, far inside the error budget).

Host does layout/encoding only: transposes, fp16 encodes of x and the shadow
half, and a phase-major permutation of the in_features axis
(p <-> 4*(p%256) + p//256) applied to both x.T and w.T. The permutation makes
each group-of-4 (the 2:4 unit) span four k-tiles at the SAME partition/column
coordinates, so sparsify+quantize is contiguous full-width elementwise ops and
the quantized weight lands directly in [in_f, out_f] (lhsT) layout. A
contraction-axis permutation applied to both operands leaves the matmul
result unchanged. All module math (threshold, mask, quantize, matmul, bias)
runs on device.

Numerics (harness gate rel_err < 2e-2; this kernel lands ~1e-3):
  s   = absmax * (1/qmax)                  (1 ulp from fl(absmax/qmax))
  inv ~= 1/s                               (HW reciprocal + 1 Newton step)
  k   = rne(w * inv)                       (magic-constant RNE round on ACT)
  q   = k * mask                           (2:4 mask; fp32 -> fp16 exact)
  y   = s * (x16 @ q.T) + bias             (scale folded into PSUM eviction)
x is fp16 (rel 2^-11); q <= 127 is fp16-exact; products exact into fp32 PSUM.
One fp16 PE pass = 1 cycle/row = the dense-matmul roofline. Threshold/mask
compares run on fp32 weights so near-tie selections match the reference.
"""

import numpy as np

N_CORES = 8
P = 128
IN_F = 1024
OUT_F = 1024
TOKENS = 32768
T_GROUPS = 4
F_GROUPS = 2
TOK_PC = TOKENS // T_GROUPS  # 8192 tokens per core
OUT_PC = OUT_F // F_GROUPS  # 512 out_features per core
K_TILES = IN_F // P  # 8
M_TILES = OUT_PC // P  # 4
TB_TOK = 1024  # token block per x strip
N_TB = TOK_PC // TB_TOK  # 8
MM_N = 512  # matmul moving free dim (one PSUM bank of fp32)
TJ = TB_TOK // MM_N  # 2

MAGIC = 12582912.0  # 1.5 * 2**23: (v + MAGIC) - MAGIC == RNE round, |v| <= 2**22

# k-tile order: range-0 tiles (phases of groups 0..127) first so the range-0
# threshold -> mask -> quant chain completes with half the weight DMA landed;
# PE accumulates k-tiles in this same order (sum order is commutative).
KT_ORDER = (0, 2, 4, 6, 1, 3, 5, 7)

N_WARM = 48  # dummy matmuls keeping the PE HAM un-throttled until real work

# phase-major permutation of the in_features axis: position p holds original
# feature 4*(p%256) + p//256, so k-tile kt covers phase kt//2 of group range
# (kt%2)*128..+128 and the four phases of a group share partition/column coords
_PERM = (4 * (np.arange(IN_F) % 256) + np.arange(IN_F) // 256).astype(np.int64)

_CACHE = {}


def _build(qmax: float):
    from contextlib import ExitStack

    import concourse.tile as tile
    import concourse.mybir as mybir
    from concourse import bacc, bass_isa

    f32 = mybir.dt.float32
    f16 = mybir.dt.float16
    Alu = mybir.AluOpType
    Act = mybir.ActivationFunctionType

    inv_qmax = float(np.float32(1.0) / np.float32(qmax))

    nc = bacc.Bacc("TRN2", target_bir_lowering=False, debug=False)
    xth = nc.dram_tensor("xth", [IN_F, TOK_PC], f16, kind="ExternalInput").ap()
    # own out_f half of w.T (permuted in_f rows), fp32: exact 2:4 tie behavior
    wpo = nc.dram_tensor("wpo", [IN_F, OUT_PC], f32, kind="ExternalInput").ap()
    # other half, fp16 shadow: only feeds the global absmax
    wpx = nc.dram_tensor("wpx", [IN_F, OUT_PC], f16, kind="ExternalInput").ap()
    biasc = nc.dram_tensor("biasc", [OUT_PC], f32, kind="ExternalInput").ap()
    yt = nc.dram_tensor("yt", [OUT_PC, TOK_PC], f16, kind="ExternalOutput").ap()

    with tile.TileContext(nc) as tc, ExitStack() as ctx:
        const = ctx.enter_context(tc.tile_pool(name="const", bufs=1))
        wnat_p = ctx.enter_context(tc.tile_pool(name="wnat", bufs=8))
        woth_p = ctx.enter_context(tc.tile_pool(name="woth", bufs=8))
        abs_p = ctx.enter_context(tc.tile_pool(name="absp", bufs=8))
        thr_p = ctx.enter_context(tc.tile_pool(name="thr", bufs=2))
        thrtmp_p = ctx.enter_context(tc.tile_pool(name="thrtmp", bufs=1))
        mask_p = ctx.enter_context(tc.tile_pool(name="mask", bufs=8))
        qtmp_p = ctx.enter_context(tc.tile_pool(name="qtmp", bufs=2))
        wqt_p = ctx.enter_context(tc.tile_pool(name="wqt", bufs=8))
        sc_p = ctx.enter_context(tc.tile_pool(name="sc", bufs=1))
        x_p = ctx.enter_context(tc.tile_pool(name="x", bufs=16))
        y_p = ctx.enter_context(tc.tile_pool(name="y", bufs=4))
        psum_mm = ctx.enter_context(tc.tile_pool(name="psmm", bufs=8, space="PSUM"))

        # ---- PE pre-warm: hold HAM at full clock until the real MMs ----
        if N_WARM > 0:
            zwarm = const.tile([P, MM_N], f16, tag="zwarm")
            nc.gpsimd.memset(zwarm[:], 0.0)
            ps_w = psum_mm.tile([P, MM_N], f32, tag="ps", name="ps_warm")
            for _ in range(N_WARM):
                nc.tensor.matmul(
                    ps_w[:], zwarm[:, 0:P], zwarm[:], start=True, stop=True
                )

        # ---- weight load (own fp32 half first, both HWDGE queues) ----
        wk = [None] * K_TILES
        ak = [None] * K_TILES
        cm = sc_p.tile([P, 2 * K_TILES], f32, tag="cm")
        for i, kt in enumerate(KT_ORDER):
            wt = wnat_p.tile([P, OUT_PC], f32, tag="wnat", name=f"wnat{kt}")
            (nc.sync if i % 2 == 0 else nc.scalar).dma_start(
                wt[:], wpo[kt * P : (kt + 1) * P, :]
            )
            wk[kt] = wt
            a = abs_p.tile([P, OUT_PC], f32, tag="abs", name=f"abs{kt}")
            nc.scalar.activation(a[:], wt[:], Act.Abs)
            ak[kt] = a
            nc.vector.tensor_reduce(
                out=cm[:, i : i + 1],
                in_=a[:],
                axis=mybir.AxisListType.X,
                op=Alu.max,
            )
        # fp16 shadow of the other half: absmax only (ACT abs + DVE max)
        for i in range(K_TILES):
            ot = woth_p.tile([P, OUT_PC], f16, tag="woth", name=f"woth{i}")
            (nc.sync if i % 2 == 0 else nc.scalar).dma_start(
                ot[:], wpx[i * P : (i + 1) * P, :]
            )
            a16 = woth_p.tile([P, OUT_PC], f16, tag="aoth", name=f"aoth{i}")
            nc.scalar.activation(a16[:], ot[:], Act.Abs)
            nc.vector.tensor_reduce(
                out=cm[:, K_TILES + i : K_TILES + i + 1],
                in_=a16[:],
                axis=mybir.AxisListType.X,
                op=Alu.max,
            )

        # ---- bias slices (scalar queue; not needed until first eviction) ----
        bias_t = []
        for mi in range(M_TILES):
            bt = const.tile([P, 1], f32, tag=f"bias{mi}")
            nc.scalar.dma_start(bt[:, 0:1], biasc[mi * P : (mi + 1) * P].unsqueeze(1))
            bias_t.append(bt)

        # ---- global absmax broadcast to all partitions ----
        amc = sc_p.tile([P, 1], f32, tag="amc")
        nc.vector.reduce_max(amc[:], cm[:], axis=mybir.AxisListType.X)
        am = sc_p.tile([P, 1], f32, tag="am")
        nc.gpsimd.partition_all_reduce(
            am[:], amc[:], channels=P, reduce_op=bass_isa.ReduceOp.max
        )

        def vts(out, in0, s1, op0, s2=None, op1=None):
            kw = {"op1": op1} if op1 is not None else {}
            nc.vector.tensor_scalar(
                out=out, in0=in0, scalar1=s1, scalar2=s2, op0=op0, **kw
            )

        def vtt(out, in0, in1, op):
            nc.vector.tensor_tensor(out=out, in0=in0, in1=in1, op=op)

        # ---- s = absmax/qmax (1 ulp); inv = 1/s (reciprocal + 1 Newton) ----
        s_t = sc_p.tile([P, 1], f32, tag="s")
        vts(s_t[:], am[:], inv_qmax, Alu.mult)
        r0 = sc_p.tile([P, 1], f32, tag="r0")
        nc.vector.reciprocal(r0[:], s_t[:])
        p1 = sc_p.tile([P, 1], f32, tag="p1")
        vtt(p1[:], s_t[:], r0[:], Alu.mult)
        e1 = sc_p.tile([P, 1], f32, tag="e1")
        vts(e1[:], p1[:], 2.0, Alu.subtract, -1.0, Alu.mult)  # 2 - s*r0
        inv_t = sc_p.tile([P, 1], f32, tag="inv")
        vtt(inv_t[:], r0[:], e1[:], Alu.mult)

        magic_t = sc_p.tile([P, 1], f32, tag="magic")
        nc.gpsimd.memset(magic_t[:], MAGIC)

        # ---- 2:4 threshold per group-range (contiguous, phases = k-tiles) ----
        # thr_r = 2nd largest |w| of each group = max(min of pair maxes,
        # max of pair mins) over the 4 phase tiles of range r
        def build_thr(r):
            a0, a1, a2, a3 = (ak[2 * j + r] for j in range(4))
            tA = thrtmp_p.tile([P, OUT_PC], f32, tag="tA", name=f"tA_{r}")
            tB = thrtmp_p.tile([P, OUT_PC], f32, tag="tB", name=f"tB_{r}")
            tC = thrtmp_p.tile([P, OUT_PC], f32, tag="tC", name=f"tC_{r}")
            tr = thr_p.tile([P, OUT_PC], f32, tag="thr", name=f"thr_{r}")
            vtt(tA[:], a0[:], a1[:], Alu.max)
            vtt(tB[:], a2[:], a3[:], Alu.max)
            vtt(tA[:], tA[:], tB[:], Alu.min)  # t1 = min of pair maxes
            vtt(tB[:], a0[:], a1[:], Alu.min)
            vtt(tC[:], a2[:], a3[:], Alu.min)
            vtt(tB[:], tB[:], tC[:], Alu.max)  # t2 = max of pair mins
            vtt(tr[:], tA[:], tB[:], Alu.max)
            return tr

        # ---- per k-tile: mask (DVE is_ge, fp32-exact ties), k = rne(w*inv)
        # (ACT magic round), q16 = (q0-MAGIC)*mask -> fp16 lhsT tile (DVE).
        wqt_by_kt = {}
        thr_cache = {}
        for kt in KT_ORDER:
            r = kt % 2
            if r not in thr_cache:
                thr_cache[r] = build_thr(r)
            wt, a, tr = wk[kt], ak[kt], thr_cache[r]
            m = mask_p.tile([P, OUT_PC], f32, tag="mask", name=f"m{kt}")
            vtt(m[:], a[:], tr[:], Alu.is_ge)

            q0 = qtmp_p.tile([P, OUT_PC], f32, tag="q0", name=f"q0_{kt}")
            # q0 = rne(w * inv) + MAGIC  (ACT: in*scale + bias)
            nc.scalar.activation(
                q0[:], wt[:], Act.Identity, bias=magic_t[:], scale=inv_t[:]
            )
            q16 = wqt_p.tile([P, OUT_PC], f16, tag="q16", name=f"q16_{kt}")
            # q16 = (q0 - MAGIC) * mask, cast fp16 (exact: integer <= qmax)
            nc.vector.scalar_tensor_tensor(
                out=q16[:], in0=q0[:], scalar=-MAGIC, in1=m[:],
                op0=Alu.add, op1=Alu.mult,
            )
            wqt_by_kt[kt] = q16
        wqt = [wqt_by_kt[kt] for kt in range(K_TILES)]

        # ---- main matmul: yt[m, t] = sum_k wqt[k,m].T @ xh[k,t] ----
        # x loads ride the sync queue; y stores ride the scalar queue
        for tb in range(N_TB):
            xh = [None] * K_TILES
            for ki in KT_ORDER:
                sl_p = slice(ki * P, (ki + 1) * P)
                sl_t = slice(tb * TB_TOK, (tb + 1) * TB_TOK)
                xht = x_p.tile([P, TB_TOK], f16, tag="xh", name=f"xh{tb}_{ki}")
                nc.sync.dma_start(xht[:], xth[sl_p, sl_t])
                xh[ki] = xht

            def evict(mi, ps_tj):
                ysb = y_p.tile([P, TB_TOK], f16, tag="ysb", name=f"y{tb}_{mi}")
                for tj in range(TJ):
                    nc.scalar.activation(
                        ysb[:, tj * MM_N : (tj + 1) * MM_N],
                        ps_tj[tj][:],
                        Act.Identity,
                        bias=bias_t[mi][:],
                        scale=s_t[:],
                    )
                tcol = tb * TB_TOK
                nc.scalar.dma_start(
                    yt[mi * P : (mi + 1) * P, tcol : tcol + TB_TOK], ysb[:]
                )

            if tb == 0:
                # k-outer sweep over all 4 m-tiles (8 PSUM banks): PE starts
                # on the first quantized k-tile, consuming at the prep pace
                ps = {
                    (mi, tj): psum_mm.tile(
                        [P, MM_N], f32, tag="ps", name=f"ps0_{mi}_{tj}"
                    )
                    for mi in range(M_TILES)
                    for tj in range(TJ)
                }
                for kpos, ki in enumerate(KT_ORDER):
                    for mi in range(M_TILES):
                        lhsT = wqt[ki][:, mi * P : (mi + 1) * P]
                        for tj in range(TJ):
                            nc.tensor.matmul(
                                ps[mi, tj][:],
                                lhsT,
                                xh[ki][:, tj * MM_N : (tj + 1) * MM_N],
                                start=(kpos == 0),
                                stop=(kpos == K_TILES - 1),
                            )
                for mi in range(M_TILES):
                    evict(mi, [ps[mi, tj] for tj in range(TJ)])
            else:
                for mi in range(M_TILES):
                    ps = [
                        psum_mm.tile(
                            [P, MM_N], f32, tag="ps", name=f"ps{tb}_{mi}_{tj}"
                        )
                        for tj in range(TJ)
                    ]
                    for kpos, ki in enumerate(KT_ORDER):
                        lhsT = wqt[ki][:, mi * P : (mi + 1) * P]
                        for tj in range(TJ):
                            nc.tensor.matmul(
                                ps[tj][:],
                                lhsT,
                                xh[ki][:, tj * MM_N : (tj + 1) * MM_N],
                                start=(kpos == 0),
                                stop=(kpos == K_TILES - 1),
                            )
                    evict(mi, ps)

    nc.compile()
    return nc


def _get(qmax: float):
    key = qmax
    if key not in _CACHE:
        _CACHE[key] = _build(qmax)
    return _CACHE[key]


def host_prep(x, weight):
    """Host-side input re-encoding: transpose, phase-major permute the in_f
    axis, fp16-encode x and the shadow halves. Pure layout/encoding."""
    xt = np.ascontiguousarray(x.T)[_PERM]  # [IN_F perm, TOKENS]
    xth = xt.astype(np.float16)
    wp = np.ascontiguousarray(weight.T[_PERM])  # [IN_F perm, OUT_F] fp32
    wp16 = wp.astype(np.float16)
    return xth, wp, wp16


LAST_EXEC_NS = None


def kernel(x, weight, bias, precision, _trace_dir=None):
    global LAST_EXEC_NS
    from concourse.bass_utils import run_bass_kernel_spmd

    x = np.asarray(x, dtype=np.float32)
    weight = np.asarray(weight, dtype=np.float32)
    bias = np.asarray(bias, dtype=np.float32)
    prec = int(np.asarray(precision))
    qmax = float(2 ** (prec - 1) - 1)

    nc = _get(qmax)

    xth, wp, wp16 = host_prep(x, weight)
    in_maps = []
    for c in range(N_CORES):
        tg, fg = c // F_GROUPS, c % F_GROUPS
        o0, o1 = fg * OUT_PC, (fg + 1) * OUT_PC
        x0, x1 = (1 - fg) * OUT_PC, (2 - fg) * OUT_PC
        in_maps.append(
            {
                "xth": np.ascontiguousarray(
                    xth[:, tg * TOK_PC : (tg + 1) * TOK_PC]
                ),
                "wpo": np.ascontiguousarray(wp[:, o0:o1]),
                "wpx": np.ascontiguousarray(wp16[:, x0:x1]),
                "biasc": np.ascontiguousarray(bias[o0:o1]),
            }
        )
    kw = {}
    if _trace_dir is not None:
        kw = {"trace": True, "tmpdir": _trace_dir}
    res = run_bass_kernel_spmd(nc, in_maps, list(range(N_CORES)), **kw)
    LAST_EXEC_NS = res.exec_time_ns
    y = np.empty((TOKENS, OUT_F), dtype=np.float32)
    for c in range(N_CORES):
        tg, fg = c // F_GROUPS, c % F_GROUPS
        y[tg * TOK_PC : (tg + 1) * TOK_PC, fg * OUT_PC : (fg + 1) * OUT_PC] = (
            res.results[c]["yt"].T.astype(np.float32)
        )
    return y
